# revision 62
# baseline (speedup 1.0000x reference)
"""DistanceAttention Trainium2 kernel.

Computes, for x:[B,T,D]:
    v    = x @ W_in.T + b_in
    attn = exp((-|i-j| + padding_mask) / e)        # [B,T,T], no softmax
    out  = attn @ v

Key facts exploited:
  * attn factors as exp(-|i-j|/e) * exp(mask_j/e).  The distance kernel
    r^|i-j| (r = exp(-1/e) ~= 0.692) underflows fp32 (< 1e-21) for
    |i-j| >= 128, so attn is numerically block-tridiagonal with three
    CONSTANT 128x128 blocks shared by every row-block/batch/core: the
    t x t matmul collapses to 3 small matmuls per 128-row block.
  * exp(mask/e) is a per-row scale of v and commutes with the
    projection: it is folded into x on the host.  Phantom halo rows are
    zero-padded, which the same mechanism handles.
  * b_in enters the output as (attn @ exp(mask/e)) (x) b_in -- a rank-1
    term added exactly on the host (b_in is zero here; generality path).

Sharding: batch(4) x seq-half(2) -> 8 cores, each owning 2048 rows plus
a 128-row halo per side.  No cross-core communication.
"""

import numpy as np

B, T, D = 4, 4096, 256
NCORES = 8
THALF = T // 2  # rows owned per core
HALO = 128
LOC = THALF + 2 * HALO  # local rows incl. halo
NBLK = LOC // 128  # 18 local 128-row blocks
# xT DMA chunk sizes in 128-row blocks: tiny first chunk unblocks the
# PE early, big chunks stream behind the first projections
CHUNKS = (2, 4, 6, 6)
NCH = len(CHUNKS)
CHOFF = tuple(sum(CHUNKS[:j]) for j in range(NCH))
E = float(np.e)

# "f32r" streams fp32 data through the PE in single-pass mode (4x the
# throughput of the 2-pass fp32 decomposition); "f32" is the safe path.
MM_DTYPE = "f32r"
SEM_POOL_STOP = 176  # shrink epilogue sem-wipe loop

_CACHE: dict = {}


def _decay_blocks() -> np.ndarray:
    """lhsT-layout decay blocks [128, 3*128]: L | 0 | R.

    matmul(out, lhsT, rhs) computes out[p,n] = sum_q lhsT[q,p] rhs[q,n].
    Out-block m needs  A_L @ v[m-1] + A_0 @ v[m] + A_R @ v[m+1]  with
      A_L[p,q] = r^(128+p-q),  A_0[p,q] = r^|p-q|,  A_R[p,q] = r^(128+q-p)
    so lhsT_L[q,p] = A_L[p,q] etc.  Entries are computed exactly like the
    reference: exp(-dist/e) in fp32.
    """
    i = np.arange(128, dtype=np.float64)
    dL = 128.0 + i[None, :] - i[:, None]  # lhsT_L[a,b] = r^(128+b-a)
    d0 = np.abs(i[:, None] - i[None, :])
    dR = 128.0 + i[:, None] - i[None, :]  # lhsT_R[a,b] = r^(128+a-b)
    dist = np.concatenate([dL, d0, dR], axis=1)
    tg = (-dist.astype(np.float32)) / np.float32(E)
    return np.exp(tg).astype(np.float32)


def _build():
    import concourse.bacc as bacc
    import concourse.mybir as mybir
    from concourse.bass import ts
    from concourse.tile import TileContext

    import concourse.bass as bass_mod

    fp = mybir.dt.float32
    mmdt = mybir.dt.float32r if MM_DTYPE == "f32r" else mybir.dt.float32

    # The kernel-end epilogue zero-resets EVERY semaphore in the kernel
    # sem range (default: the whole 254-sem bank, ~50 resets per engine,
    # ~7us with Tensor the slowest).  This kernel uses ~14 sems; shrink
    # the reserved range so the wipe loop is proportionally shorter.
    orig_range = bass_mod.get_kernel_semaphore_range()
    bass_mod.get_kernel_semaphore_range = lambda: range(
        orig_range.start, min(orig_range.stop, SEM_POOL_STOP))
    try:
        nc = bacc.Bacc(None, target_bir_lowering=False, debug=False)
    finally:
        bass_mod.get_kernel_semaphore_range = lambda: orig_range

    # host-packed streams: "head" carries W.T halves + first x chunk +
    # decay blocks in ONE transfer; xc{j} carry the remaining x chunks
    # with both d-halves side by side.  One DMA each, plain 2D.
    head = nc.dram_tensor("head", [128, 2 * D + 2 * CHUNKS[0] * 128 + 384], mmdt,
                          kind="ExternalInput")
    xcd = [None] * NCH
    for j in range(1, NCH):
        xcd[j] = nc.dram_tensor(f"xc{j}", [128, 2 * CHUNKS[j] * 128], mmdt,
                                kind="ExternalInput")
    out = nc.dram_tensor("out", [THALF, D], fp, kind="ExternalOutput")

    with TileContext(nc) as tc:
        with (
            tc.tile_pool(name="const", bufs=1) as cpool,
            tc.tile_pool(name="vpool", bufs=1) as vpool,
            tc.tile_pool(name="opool", bufs=3) as opool,
            tc.tile_pool(name="ppsum", bufs=3, space="PSUM") as ppsum,
            tc.tile_pool(name="dpsum", bufs=4, space="PSUM") as dpsum,
        ):
            # PE warmup: dummy matmuls with no data deps run during the
            # DMA lead so the HAM clock gate is at 8/8 (2.4 GHz) by the
            # time the first real matmul issues (~3.4us busy to warm)
            scr_w = cpool.tile([128, 128], fp, tag="scr_w")
            nc.vector.memset(scr_w[:], 0.0)
            scr_x = cpool.tile([128, 2 * D], fp, tag="scr_x")
            nc.vector.memset(scr_x[:], 0.0)
            wpsum = ppsum.tile([128, 2 * D], fp, tag="warm", bufs=1)
            for _ in range(3):
                nc.tensor.matmul(wpsum[:], scr_w[:], scr_x[:],
                                 start=True, stop=True)

            # DMA order = dependency order of the first matmuls; all on
            # one HWDGE queue -- serial issue naturally prioritizes the
            # early critical transfers over the big later chunks (a
            # parallel-queue split was measured slower: every transfer
            # then contends for HBM bandwidth at once).  Host-side
            # packing turns all of w/x0/md into ONE plain 2D DMA and
            # each later chunk into one more, minimizing the ~650ns
            # per-DMA issue serialization.
            x0w = 2 * CHUNKS[0] * 128
            head_sb = cpool.tile([128, 2 * D + x0w + 384], mmdt, tag="head")
            nc.sync.dma_start(out=head_sb[:], in_=head[:])
            wT_sb = [head_sb[:, 0:D], head_sb[:, D:2 * D]]
            xb = [None] * NCH
            xb[0] = head_sb[:, 2 * D:2 * D + x0w]
            md_sb = head_sb[:, 2 * D + x0w:2 * D + x0w + 384]
            for j in range(1, NCH):
                t = cpool.tile([128, 2 * CHUNKS[j] * 128], mmdt,
                               name=f"xb{j}", tag=f"xb{j}")
                nc.sync.dma_start(out=t[:], in_=xcd[j][:])
                xb[j] = t

            # all 18 v blocks in one tile so any 512-wide window
            # [v_a | v_a+1] is a contiguous rhs
            v_sb = vpool.tile([128, NBLK * D], mmdt, tag="v")
            # single output staging tile + manually-reused PSUM tiles:
            # fewer tile allocations shrink the kernel-tail release
            # protocol (~115ns of sem traffic per allocation per engine)
            o_sb = opool.tile([128, 8 * 2 * D], fp, tag="o")
            pps = [ppsum.tile([128, 2 * D], fp, name=f"pp{i}", tag=f"pp{i}",
                              bufs=1) for i in range(3)]
            dps = [dpsum.tile([128, 2 * D], fp, name=f"dp{i}", tag=f"dp{i}",
                              bufs=1) for i in range(4)]

            def xap(k, m):  # lhsT for t-block m, d-half k
                j = max(jj for jj in range(NCH) if CHOFF[jj] <= m)
                return xb[j][:, ts(k * CHUNKS[j] + m - CHOFF[j], 128)]

            def proj_pair(p):
                # project blocks (2p, 2p+1) into one [128, 512] PSUM pair
                a = 2 * p
                pp = pps[p % 3]
                nc.tensor.matmul(pp[:, 0:D], xap(0, a), wT_sb[0][:],
                                 start=True, stop=False)
                nc.tensor.matmul(pp[:, D:2 * D], xap(0, a + 1), wT_sb[0][:],
                                 start=False, stop=False)
                nc.tensor.matmul(pp[:, 0:D], xap(1, a), wT_sb[1][:],
                                 start=False, stop=False)
                nc.tensor.matmul(pp[:, D:2 * D], xap(1, a + 1), wT_sb[1][:],
                                 start=False, stop=True)
                nc.vector.tensor_copy(v_sb[:, a * D:(a + 2) * D], pp[:])

            def decay_pair(a, copy_eng=None):
                # out blocks (a, a+1) as one [128, 512] PSUM pair:
                # each diagonal's weights apply to both halves at once
                dp = dps[((a - 1) // 2) % 4]
                nc.tensor.matmul(dp[:], md_sb[:, 0:128],
                                 v_sb[:, (a - 1) * D:(a + 1) * D],
                                 start=True, stop=False)
                nc.tensor.matmul(dp[:], md_sb[:, 128:256],
                                 v_sb[:, a * D:(a + 2) * D],
                                 start=False, stop=False)
                nc.tensor.matmul(dp[:], md_sb[:, 256:384],
                                 v_sb[:, (a + 1) * D:(a + 3) * D],
                                 start=False, stop=True)
                dst = out.rearrange("(n p) d -> p n d", p=128)[:, a - 1:a + 1, :]
                ob = o_sb[:, (a - 1) * D:(a + 1) * D]
                if a == NBLK - 3:
                    # last pair: halve the terminal copy+DMA chain by
                    # running both halves on both engines in parallel
                    nc.vector.tensor_copy(ob[:, 0:D], dp[:, 0:D])
                    nc.scalar.copy(ob[:, D:2 * D], dp[:, D:2 * D])
                    nc.sync.dma_start(out=dst[:, 0:1, :],
                                      in_=ob[:, 0:D].unsqueeze(1))
                    nc.sync.dma_start(out=dst[:, 1:2, :],
                                      in_=ob[:, D:2 * D].unsqueeze(1))
                else:
                    # out-copies alternate between the otherwise-idle
                    # scalar engine and the DVE so non-critical copies
                    # stay out of the DVE cast stream
                    copy_eng(ob, dp[:])
                    nc.sync.dma_start(
                        out=dst, in_=ob.rearrange("p (n d) -> p n d", n=2))

            # interleave: decay pair a=2k+1 (v blocks a-1..a+2) becomes
            # ready right after proj pair k+1 -- emit it there so its
            # copy/DMA drain while later projections still run
            proj_pair(0)
            proj_pair(1)
            decay_pair(1, nc.scalar.copy)
            for p in range(2, NBLK // 2):
                proj_pair(p)
                decay_pair(2 * p - 1,
                           nc.scalar.copy if p % 2 else nc.vector.tensor_copy)

    nc.compile()
    return nc


def _shard_inputs(x, padding_mask, W_in, b_in):
    x = np.asarray(x, np.float32)
    padding_mask = np.asarray(padding_mask, np.float32)
    if np.any(padding_mask):
        x = x * np.exp(padding_mask / np.float32(E)).transpose(0, 2, 1)
    wT = np.asarray(W_in, np.float32).T.reshape(2, 128, D)
    wpack = np.concatenate([wT[0], wT[1]], axis=1)  # [128, 2D]
    mdec = _decay_blocks()
    in_maps = []
    for c in range(NCORES):
        bidx, half = divmod(c, 2)
        start = half * THALF
        lo, hi = start - HALO, start + THALF + HALO
        glo, ghi = max(lo, 0), min(hi, T)
        xsl = np.zeros((LOC, D), np.float32)
        xsl[glo - lo:ghi - lo] = x[bidx, glo:ghi]
        xTc = xsl.T.reshape(2, 128, LOC)

        def chunk(j):  # [128, 2*cols]: both d-halves side by side
            c0, c1 = CHOFF[j] * 128, (CHOFF[j] + CHUNKS[j]) * 128
            return np.concatenate([xTc[0][:, c0:c1], xTc[1][:, c0:c1]], axis=1)

        im = {"head": np.ascontiguousarray(
            np.concatenate([wpack, chunk(0), mdec], axis=1))}
        for j in range(1, NCH):
            im[f"xc{j}"] = np.ascontiguousarray(chunk(j))
        in_maps.append(im)
    return in_maps


def _bias_correction(out, padding_mask, b_in):
    """out += attn @ (1 (x) b_in) = (attn_dist @ exp(mask/e)) (x) b_in."""
    b_in = np.asarray(b_in, np.float32)
    if not np.any(b_in):
        return
    k = np.arange(-256, 257, dtype=np.float32)
    w = np.exp(-np.abs(k) / np.float32(E)).astype(np.float64)
    s_all = np.exp(np.asarray(padding_mask, np.float32)[:, 0, :]
                   / np.float32(E)).astype(np.float64)
    for bidx in range(B):
        a = np.convolve(s_all[bidx], w, mode="same").astype(np.float32)
        out[bidx] += np.outer(a, b_in)


def kernel(x, padding_mask, W_in, b_in):
    from concourse.bass_utils import run_bass_kernel_spmd

    if "nc" not in _CACHE:
        _CACHE["nc"] = _build()
    nc = _CACHE["nc"]

    in_maps = _shard_inputs(x, padding_mask, W_in, b_in)
    res = run_bass_kernel_spmd(nc, in_maps, list(range(NCORES)))
    out = np.empty((B, T, D), np.float32)
    for c in range(NCORES):
        bidx, half = divmod(c, 2)
        out[bidx, half * THALF:(half + 1) * THALF] = res.results[c]["out"]
    _bias_correction(out, padding_mask, b_in)
    return out
